# revision 28
# baseline (speedup 1.0000x reference)
"""Multi-head attention (B=8, N=1024, D=512, H=8) on 8 TRN2 NeuronCores.

Sharding: pure batch-parallel - core i computes batch i end-to-end, no
collectives. Host-side prep per batch: gather valid keys (mask) into a
contiguous buffer padded to NKV=640, pre-transpose x, convert streams to
bf16, and pack all device inputs into 5 grouped DRAM tensors so the whole
input set loads with 5 large dma_starts issued in priority order on one
queue (sequential completion: kproj inputs land first).

Device pipeline (bf16 matmuls, f32 PSUM):
  PE warmup (dummy matmuls on zeros during the DMA flight, so the HAM
  clock gate opens before real work) -> k projection -> q projection for
  head-pair 0 -> attention over head pairs. Scores for heads (2dp,2dp+1)
  land in one [128,1024] PSUM tile per (chunk, query-half) via row-packed
  matmul pairs on disjoint PE row groups; one exp per tile with the
  key-padding mask folded into the activation bias; attn@v as col-packed
  M=64 matmul pairs into one [128,512] PSUM tile; softmax denominators
  via four concurrent M=1 ones-matmuls into one shared PSUM bank.
  Remaining projections (v, q for pairs 1-3) are emitted as fillers
  inside the attention loop to use PE slack while the ACT engine (the
  bottleneck: 40 exps) streams. Normalize = reciprocal + gpsimd
  partition-broadcast + one DVE multiply per (pair, half). Tail: output
  projection with bf16 DMA writeback (host upcasts).

Math shortcuts: bk dropped (cancels in softmax); bv folded into the
output bias on the host (bob' = bo + bv @ wo).
"""

import sys

import numpy as np

sys.path.insert(0, "/opt/trn_rl_repo")

B, N, D, H = 8, 1024, 512, 8
HD = D // H            # 64
SCALE = HD ** -0.5     # 0.125
NKV = 640              # padded valid-key count (5 chunks of 128)
KC = NKV // 128        # 5
DC = D // 128          # 4
PAD_BIAS = -30000.0    # exp(PAD_BIAS + s*SCALE) == 0.0 exactly

# g4 column layout (f32): bq columns, expb columns, bob block
G4_BQ = 0
G4_EXPB = DC
G4_BOB = DC + KC
G4_W = DC + KC + D

_DEBUG = False  # extra dbg output tensor with intermediate probes
_NPROBE = 10

_prog_cache = {}


def _build_program():
    import concourse.bacc as bacc
    import concourse.tile as tile
    from concourse import mybir

    dt = mybir.dt
    f32 = dt.float32
    bf16 = dt.bfloat16
    AF = mybir.ActivationFunctionType

    nc = bacc.Bacc("TRN2", target_bir_lowering=False, debug=False)

    # grouped inputs (one dma_start each, issued in priority order)
    g0_d = nc.dram_tensor("g0", [128, DC * NKV + DC * D], bf16,
                          kind="ExternalInput").ap()
    g1_d = nc.dram_tensor("g1", [128, DC * N + DC * D], bf16, kind="ExternalInput").ap()
    g2_d = nc.dram_tensor("g2", [128, DC * D + D], bf16,
                      kind="ExternalInput").ap()
    g3_d = nc.dram_tensor("g3", [128, DC * D], bf16, kind="ExternalInput").ap()
    g4_d = nc.dram_tensor("g4", [128, G4_W], f32, kind="ExternalInput").ap()
    y_d = nc.dram_tensor("y", [N, D], bf16, kind="ExternalOutput").ap()
    dbg_d = (nc.dram_tensor("dbg", [128, 512 * _NPROBE], f32,
                            kind="ExternalOutput").ap() if _DEBUG else None)

    with tile.TileContext(nc) as tc, \
         nc.allow_low_precision(reason="bf16 matmul streams, f32 accumulate"):
        with tc.tile_pool(name="const", bufs=1) as cpool:
            G0 = cpool.tile([128, DC * NKV + DC * D], bf16, name="G0")
            G1 = cpool.tile([128, DC * N + DC * D], bf16, name="G1")
            G2 = cpool.tile([128, DC * D + D], bf16, name="G2")
            G3 = cpool.tile([128, DC * D], bf16, name="G3")
            G4 = cpool.tile([128, G4_W], f32, name="G4")

            def xkT(c2):  # [128, NKV] slice for D-chunk c2
                return G0[:, NKV * c2:NKV * (c2 + 1)]

            def wk(c2):   # [128, D]
                return G0[:, DC * NKV + D * c2:DC * NKV + D * (c2 + 1)]

            def xT(c2):   # [128, N]
                return G1[:, N * c2:N * (c2 + 1)]

            def wq(c2):   # [128, D]
                return G1[:, DC * N + D * c2:DC * N + D * (c2 + 1)]

            def wv(c2):
                return G2[:, D * c2:D * (c2 + 1)]

            def wo(c2):
                return G3[:, D * c2:D * (c2 + 1)]

            # persistent result tiles
            kT_t = [cpool.tile([128, NKV], bf16, name=f"kT{c}")
                    for c in range(DC)]
            qT_t = [cpool.tile([128, N], bf16, name=f"qT{c}")
                    for c in range(DC)]
            v_t = [cpool.tile([128, D], bf16, name=f"v{c}")
                   for c in range(KC)]
            aoT_t = [cpool.tile([128, N], bf16, name=f"aoT{c}")
                     for c in range(DC)]
            zeros = cpool.tile([128, 512], bf16, name="zeros")
            ones_kv = cpool.tile([128, 1], bf16, name="ones_kv")
            onesq = cpool.tile([1, 128], bf16, name="onesq")
            ones97 = cpool.tile([97, 64], bf16, name="ones97")
            dum = cpool.tile([1, 32], f32, name="dum")
            dbg = (cpool.tile([128, 512 * _NPROBE], f32, name="dbg")
                   if _DEBUG else None)

            def probe(k, src, rows=128, cols=512, row0=0):
                if _DEBUG:
                    r1 = row0 + 1 if rows is None else rows
                    nc.vector.tensor_scalar_add(
                        dbg[row0:r1, 512 * k:512 * k + cols], src, 0.0)

            # ---- issue everything up front ----
            nc.vector.memset(zeros[:], 0.0)
            nc.vector.memset(ones_kv[:], 1.0)
            nc.vector.memset(onesq[:], 1.0)
            nc.vector.memset(ones97[:], 1.0)
            if _DEBUG:
                nc.vector.memset(dbg[:], 0.0)
            nc.sync.dma_start(G0[:], g0_d[:, :])
            nc.sync.dma_start(G1[:], g1_d[:, :])
            nc.sync.dma_start(G2[:], g2_d[:, :])
            nc.sync.dma_start(G3[:], g3_d[:, :])
            nc.sync.dma_start(G4[:], g4_d[:, :])
            # preload the exp table set while DMA is in flight
            nc.scalar.activation(dum[:], zeros[0:1, 0:32], AF.Exp, scale=1.0)

            # ---- PE warmup: open the HAM clock gate during DMA flight ----
            with tc.tile_pool(name="wp", bufs=1, space="PSUM") as wp:
                wps = wp.tile([128, 512], f32, name="wps")
                for _ in range(32):
                    nc.tensor.matmul(wps[:, 0:128], zeros[:, 0:128], zeros[:, 0:128],
                                     start=True, stop=True)

            # ---- k projection (no bias: cancels in softmax) ----
            with tc.tile_pool(name="kpp", bufs=2, space="PSUM") as kpp:
                for dp in range(DC):
                    ps = kpp.tile([128, NKV], f32, name="kps")
                    for c2 in range(DC):
                        lhs = wk(c2)[:, 128 * dp:128 * (dp + 1)]
                        nc.tensor.matmul(
                            ps[:, 0:512], lhs, xkT(c2)[:, 0:512],
                            start=(c2 == 0), stop=(c2 == DC - 1))
                        nc.tensor.matmul(
                            ps[:, 512:NKV], lhs, xkT(c2)[:, 512:NKV],
                            start=(c2 == 0), stop=(c2 == DC - 1))
                    nc.scalar.copy(kT_t[dp][:], ps[:])

            # ---- q projection for head pair 0 (rest are fillers) ----
            with tc.tile_pool(name="qpp", bufs=2, space="PSUM") as qpp:
                qps = [qpp.tile([128, 512], f32, name="qps")
                       for hf in range(2)]
                for c2 in range(DC):
                    for hf in range(2):
                        nc.tensor.matmul(
                            qps[hf][:], wq(c2)[:, 0:128],
                            xT(c2)[:, 512 * hf:512 * (hf + 1)],
                            start=(c2 == 0), stop=(c2 == DC - 1))
                for hf in range(2):
                    nc.vector.tensor_scalar_add(
                        qT_t[0][:, 512 * hf:512 * (hf + 1)], qps[hf][:],
                        G4[:, G4_BQ:G4_BQ + 1])

            # ---- v projection for chunks 0..3 (c4 is a filler) ----
            with tc.tile_pool(name="vpp", bufs=2, space="PSUM") as vpp:
                for c in range(KC - 1):
                    ps = vpp.tile([128, 512], f32, name="vps")
                    for c2 in range(DC):
                        nc.tensor.matmul(
                            ps[:], xkT(c2)[:, 128 * c:128 * (c + 1)],
                            wv(c2)[:], start=(c2 == 0), stop=(c2 == DC - 1))
                    nc.vector.tensor_scalar_add(v_t[c][:], ps[:], 0.0)

            # ---- attention over head pairs, leftover proj as fillers ----
            with tc.tile_pool(name="scp", bufs=2, space="PSUM") as scp, \
                 tc.tile_pool(name="oap", bufs=2, space="PSUM") as oap, \
                 tc.tile_pool(name="dnp", bufs=1, space="PSUM") as dnp, \
                 tc.tile_pool(name="flp", bufs=1, space="PSUM") as flp, \
                 tc.tile_pool(name="pp", bufs=4) as pp, \
                 tc.tile_pool(name="rcp", bufs=4) as rcp:

                # filler units emit 2 matmuls per step so one step fits the
                # PE slack of a chunk slot; the final step adds the
                # PSUM->SBUF copy
                class FillV:
                    def __init__(self, c):
                        self.c, self.ps = c, None

                    def step(self, half):
                        if half == 0:
                            self.ps = flp.tile([128, 512], f32, name="fps")
                        for c2 in (0, 1) if half == 0 else (2, 3):
                            nc.tensor.matmul(
                                self.ps[:],
                                xkT(c2)[:, 128 * self.c:128 * (self.c + 1)],
                                wv(c2)[:], start=(c2 == 0), stop=(c2 == 3))
                        if half == 1:
                            nc.vector.tensor_scalar_add(
                                v_t[self.c][:], self.ps[:], 0.0)

                class FillQ:
                    def __init__(self, dp, hf):
                        self.dp, self.hf, self.ps = dp, hf, None

                    def step(self, half):
                        if half == 0:
                            self.ps = flp.tile([128, 512], f32, name="fps")
                        for c2 in (0, 1) if half == 0 else (2, 3):
                            nc.tensor.matmul(
                                self.ps[:],
                                wq(c2)[:, 128 * self.dp:128 * (self.dp + 1)],
                                xT(c2)[:, 512 * self.hf:512 * (self.hf + 1)],
                                start=(c2 == 0), stop=(c2 == 3))
                        if half == 1:
                            nc.vector.tensor_scalar_add(
                                qT_t[self.dp][:,
                                              512 * self.hf:512 * (self.hf + 1)],
                                self.ps[:],
                                G4[:, G4_BQ + self.dp:G4_BQ + self.dp + 1])

                v4 = FillV(4)
                q10, q11 = FillQ(1, 0), FillQ(1, 1)
                q20, q21 = FillQ(2, 0), FillQ(2, 1)
                q30, q31 = FillQ(3, 0), FillQ(3, 1)
                fillers = {
                    (0, 0): [(v4, 0), (v4, 1)],
                    (0, 1): [(q10, 0)], (0, 2): [(q10, 1)],
                    (0, 3): [(q11, 0)], (0, 4): [(q11, 1)],
                    (1, 0): [(q20, 0)], (1, 1): [(q20, 1)],
                    (1, 2): [(q21, 0)], (1, 3): [(q21, 1)],
                    (1, 4): [(q30, 0)],
                    (2, 0): [(q30, 1)], (2, 1): [(q31, 0)],
                    (2, 2): [(q31, 1)],
                }

                pending_norm = []
                for dp in range(DC):
                    oa = [oap.tile([128, 512], f32, name="oa")
                          for hf in range(2)]
                    den = dnp.tile([128, 512], f32, name="den")
                    p_t = []  # per chunk: [p_hf0, p_hf1]

                    def scores4(c):
                        # 4 matmuls sharing one kT weight pair: the two
                        # query halves reuse the loaded weights, the A/B
                        # heads run concurrently on disjoint PE row groups
                        sc = [scp.tile([128, N], f32, name="sc")
                              for hf in range(2)]
                        for hf in range(2):
                            for hi in range(2):
                                nc.tensor.matmul(
                                    sc[hf][:, 512 * hi:512 * (hi + 1)],
                                    kT_t[dp][HD * hi:HD * (hi + 1),
                                             128 * c:128 * (c + 1)],
                                    qT_t[dp][HD * hi:HD * (hi + 1),
                                             512 * hf:512 * (hf + 1)],
                                    start=True, stop=True)
                        ps = []
                        for hf in range(2):
                            p = pp.tile([128, N], bf16, name="p")
                            nc.scalar.activation(
                                p[:], sc[hf][:], AF.Exp,
                                bias=G4[:, G4_EXPB + c:G4_EXPB + c + 1],
                                scale=SCALE)
                            ps.append(p)
                        if _DEBUG and dp == 0 and c == 0:
                            probe(5, sc[0][:, 0:512])
                            probe(6, ps[0][:, 0:512])
                        return ps, sc

                    def av4den4(c):
                        # col-packed pairs: head A -> partitions 0:64, head
                        # B -> 64:128 of one PSUM bank per query half; both
                        # halves adjacent so the v weights load once. Each
                        # writer is its own accumulation group over c
                        # (per-partition zero regions) -> skip_group_check.
                        for hf in range(2):
                            for hi in range(2):
                                h = 2 * dp + hi
                                nc.tensor.matmul(
                                    oa[hf][64 * hi:64 * (hi + 1), :],
                                    v_t[c][:, 64 * h:64 * h + 64],
                                    p_t[c][hf][:, 512 * hi:512 * (hi + 1)],
                                    start=(c == 0), stop=(c == KC - 1),
                                    tile_position=(0, 64 * hi),
                                    skip_group_check=True)
                        # denominators: M=1 ones-matmuls on 4 distinct col
                        # groups run concurrently off one weight load
                        for hf in range(2):
                            for hi in range(2):
                                r = 64 * hf + 32 * hi
                                nc.tensor.matmul(
                                    den[r:r + 1, :], ones_kv[:],
                                    p_t[c][hf][:, 512 * hi:512 * (hi + 1)],
                                    start=(c == 0), stop=(c == KC - 1),
                                    tile_position=(0, r),
                                    skip_group_check=True)

                    def keep_warm(sc_dead, n):
                        # dead-tile dummy matmuls: keep the PE busy across
                        # cross-engine gate windows so the HAM clock gate
                        # stays open (micro-idle re-throttles cost 2x)
                        for _ in range(n):
                            nc.tensor.matmul(
                                sc_dead[:, 0:256], zeros[:, 0:128],
                                zeros[:, 0:256], start=True, stop=True)

                    sc_t = []
                    for c in range(KC):
                        pc, scc = scores4(c)
                        p_t.append(pc)
                        sc_t.append(scc)
                        if c == 0 and pending_norm:
                            keep_warm(prev_sc1, 16)
                            pending_norm.pop()()
                        if c >= 1:
                            av4den4(c - 1)
                        for unit, half in fillers.get((dp, c), ()):
                            unit.step(half)
                    av4den4(KC - 1)
                    prev_sc1 = sc_t[KC - 1][1]
                    if dp == DC - 1:
                        keep_warm(prev_sc1, 26)

                    # normalize: stage the four denominator rows into SBUF
                    # (same-partition copies; SBUF APs must start at
                    # 0/32/64/96), one reciprocal from partition 0 (custom
                    # DVE ops misbehave at non-zero start partitions), then
                    # partition-broadcast via col-packed K=1 matmuls
                    # (gpsimd partition_broadcast cannot write partitions
                    # 64:128 on HW) and one multiply per query half
                    def normalize(dp=dp, oa=oa, den=den):
                        db = rcp.tile([97, 512], f32, name="db")
                        rcf = rcp.tile([97, 512], f32, name="rcf")
                        rc = rcp.tile([97, 512], bf16, name="rc")
                        nc.vector.memset(db[:], 1.0)
                        for j in range(4):
                            r = 32 * j
                            nc.vector.tensor_scalar_add(
                                db[r:r + 1, :], den[r:r + 1, :], 0.0)
                        nc.vector.reciprocal_approx_fast(rcf[:], db[:])
                        nc.vector.tensor_scalar_add(rc[:], rcf[:], 0.0)
                        if _DEBUG and dp == 0:
                            for j in range(4):
                                r = 32 * j
                                probe(0, db[r:r + 1, :], rows=None, row0=r)
                                probe(1, rc[r:r + 1, :], rows=None, row0=r)
                        for hf in range(2):
                            rbs = flp.tile([128, 512], f32, name="fps")
                            for hi in range(2):
                                r = 64 * hf + 32 * hi
                                nc.tensor.matmul(
                                    rbs[64 * hi:64 * (hi + 1), :],
                                    ones97[r:r + 1, :], rc[r:r + 1, :],
                                    start=True, stop=True,
                                    tile_position=(r, 64 * hi),
                                    skip_group_check=True)
                            rbs_sb = rcp.tile([128, 512], f32, name="rbs_sb")
                            nc.vector.tensor_scalar_add(rbs_sb[:], rbs[:], 0.0)
                            if _DEBUG and dp == 0 and hf == 0:
                                probe(2, rbs_sb[:])
                                probe(3, oa[0][:])
                            nc.vector.tensor_mul(
                                aoT_t[dp][:, 512 * hf:512 * (hf + 1)],
                                oa[hf][:], rbs_sb[:])
                        if _DEBUG and dp == 0:
                            probe(4, aoT_t[0][:, 0:512])
                            probe(7, kT_t[0][:, 0:512])
                            probe(8, qT_t[0][:, 0:512])
                            probe(9, v_t[0][:, 0:512])

                    if dp == DC - 1:
                        normalize()
                    else:
                        pending_norm.append(normalize)

            # ---- output projection, bf16 writeback ----
            with tc.tile_pool(name="ypp", bufs=2, space="PSUM") as ypp, \
                 tc.tile_pool(name="ysp", bufs=2) as ysp:
                for ic in range(N // 128):
                    yps = ypp.tile([128, D], f32, name="yps")
                    for dp in range(DC):
                        nc.tensor.matmul(
                            yps[:], aoT_t[dp][:, 128 * ic:128 * (ic + 1)],
                            wo(dp)[:], start=(dp == 0), stop=(dp == DC - 1))
                    ysb = ysp.tile([128, D], bf16, name="ysb")
                    nc.vector.tensor_add(ysb[:], yps[:],
                                         G4[:, G4_BOB:G4_BOB + D])
                    nc.sync.dma_start(y_d[128 * ic:128 * (ic + 1), :], ysb[:])
                if _DEBUG:
                    nc.sync.dma_start(dbg_d[:, :], dbg[:])

    return nc


def _get_program():
    if "nc" not in _prog_cache:
        nc = _build_program()
        if not nc.is_finalized():
            nc.finalize()
        _prog_cache["nc"] = nc
    return _prog_cache["nc"]


def _prep_core(b, x, mask, wq, bq, wk, bk, wv, bv, wo, bo):
    import ml_dtypes

    b16 = ml_dtypes.bfloat16
    f = np.float32
    xb = np.ascontiguousarray(x[b], dtype=f)                # [N, D]
    idx = np.nonzero(mask[b])[0]
    nv = int(idx.size)
    assert 1 <= nv <= NKV, f"batch {b}: {nv} valid keys, NKV={NKV}"
    xk = np.zeros((NKV, D), f)
    xk[:nv] = xb[idx]
    xkT = np.ascontiguousarray(xk.T)                        # [D, NKV]
    xT = np.ascontiguousarray(xb.T)                         # [D, N]

    def chunks(a):  # [D, W] -> [128, DC*W]
        return np.concatenate([a[128 * c:128 * (c + 1), :]
                               for c in range(DC)], axis=1)

    g0 = np.concatenate([chunks(xkT), chunks(wk.astype(f))], axis=1)
    g1 = np.concatenate([chunks(xT), chunks(wq.astype(f))], axis=1)
    bobv = (bo.astype(f) + bv.astype(f) @ wo.astype(f)).reshape(1, D)
    g2 = np.concatenate([chunks(wv.astype(f)),
                         np.broadcast_to(bobv, (128, D))], axis=1)
    g3 = chunks(wo.astype(f))

    pos = np.arange(128)[:, None] + 128 * np.arange(KC)[None, :]
    expb = np.where(pos < nv, 0.0, PAD_BIAS).astype(f)      # [128, KC]
    bqc = np.stack([bq.astype(f)[128 * c:128 * (c + 1)]
                    for c in range(DC)], axis=1)            # [128, DC]
    bob = (bo.astype(f) + bv.astype(f) @ wo.astype(f)).reshape(D)
    g4 = np.concatenate([bqc, expb,
                         np.broadcast_to(bob, (128, D))], axis=1)
    return {
        "g0": np.ascontiguousarray(g0).astype(b16),
        "g1": np.ascontiguousarray(g1).astype(b16),
        "g2": np.ascontiguousarray(g2).astype(b16),
        "g3": np.ascontiguousarray(g3).astype(b16),
        "g4": np.ascontiguousarray(g4, f),
    }


def _run(inputs):
    import os

    os.environ["BASS_NEVER_TRACE"] = "1"
    from concourse.bass_utils import run_bass_kernel_spmd

    nc = _get_program()
    in_maps = [_prep_core(b, **inputs) for b in range(B)]
    res = run_bass_kernel_spmd(nc, in_maps, core_ids=list(range(B)),
                               trace=False)
    out = np.stack([res.results[b]["y"] for b in range(B)], axis=0)
    return out.astype(np.float32), res


def kernel(**inputs) -> np.ndarray:
    out, _ = _run(inputs)
    return out


# revision 29
# speedup vs baseline: 1.0025x; 1.0025x over previous
"""Multi-head attention (B=8, N=1024, D=512, H=8) on 8 TRN2 NeuronCores.

Sharding: pure batch-parallel - core i computes batch i end-to-end, no
collectives. Host-side prep per batch: gather valid keys (mask) into a
contiguous buffer padded to NKV=640, pre-transpose x, convert streams to
bf16, and pack all device inputs into 5 grouped DRAM tensors so the whole
input set loads with 5 large dma_starts issued in priority order on one
queue (sequential completion: kproj inputs land first).

Device pipeline (bf16 matmuls, f32 PSUM):
  PE warmup (dummy matmuls on zeros during the DMA flight, so the HAM
  clock gate opens before real work) -> k projection -> q projection for
  head-pair 0 -> attention over head pairs. Scores for heads (2dp,2dp+1)
  land in one [128,1024] PSUM tile per (chunk, query-half) via row-packed
  matmul pairs on disjoint PE row groups; one exp per tile with the
  key-padding mask folded into the activation bias; attn@v as col-packed
  M=64 matmul pairs into one [128,512] PSUM tile; softmax denominators
  via four concurrent M=1 ones-matmuls into one shared PSUM bank.
  Remaining projections (v, q for pairs 1-3) are emitted as fillers
  inside the attention loop to use PE slack while the ACT engine (the
  bottleneck: 40 exps) streams. Normalize = reciprocal + gpsimd
  partition-broadcast + one DVE multiply per (pair, half). Tail: output
  projection with bf16 DMA writeback (host upcasts).

Math shortcuts: bk dropped (cancels in softmax); bv folded into the
output bias on the host (bob' = bo + bv @ wo).
"""

import sys

import numpy as np

sys.path.insert(0, "/opt/trn_rl_repo")

B, N, D, H = 8, 1024, 512, 8
HD = D // H            # 64
SCALE = HD ** -0.5     # 0.125
NKV = 640              # padded valid-key count (5 chunks of 128)
KC = NKV // 128        # 5
DC = D // 128          # 4
PAD_BIAS = -30000.0    # exp(PAD_BIAS + s*SCALE) == 0.0 exactly

# g4 column layout (f32): bq columns, expb columns, bob block
G4_BQ = 0
G4_EXPB = DC
G4_BOB = DC + KC
G4_W = DC + KC + D

_DEBUG = False  # extra dbg output tensor with intermediate probes
_NPROBE = 10

_prog_cache = {}


def _build_program():
    import concourse.bacc as bacc
    import concourse.tile as tile
    from concourse import mybir

    dt = mybir.dt
    f32 = dt.float32
    bf16 = dt.bfloat16
    AF = mybir.ActivationFunctionType

    nc = bacc.Bacc("TRN2", target_bir_lowering=False, debug=False)

    # grouped inputs (one dma_start each, issued in priority order)
    g0_d = nc.dram_tensor("g0", [128, DC * NKV + DC * D], bf16,
                          kind="ExternalInput").ap()
    g1_d = nc.dram_tensor("g1", [128, DC * N + DC * D], bf16, kind="ExternalInput").ap()
    g2_d = nc.dram_tensor("g2", [128, DC * D + D], bf16,
                      kind="ExternalInput").ap()
    g3_d = nc.dram_tensor("g3", [128, DC * D], bf16, kind="ExternalInput").ap()
    g4_d = nc.dram_tensor("g4", [128, G4_W], f32, kind="ExternalInput").ap()
    y_d = nc.dram_tensor("y", [N, D], bf16, kind="ExternalOutput").ap()
    dbg_d = (nc.dram_tensor("dbg", [128, 512 * _NPROBE], f32,
                            kind="ExternalOutput").ap() if _DEBUG else None)

    with tile.TileContext(nc) as tc, \
         nc.allow_low_precision(reason="bf16 matmul streams, f32 accumulate"):
        with tc.tile_pool(name="const", bufs=1) as cpool:
            G0 = cpool.tile([128, DC * NKV + DC * D], bf16, name="G0")
            G1 = cpool.tile([128, DC * N + DC * D], bf16, name="G1")
            G2 = cpool.tile([128, DC * D + D], bf16, name="G2")
            G3 = cpool.tile([128, DC * D], bf16, name="G3")
            G4 = cpool.tile([128, G4_W], f32, name="G4")

            def xkT(c2):  # [128, NKV] slice for D-chunk c2
                return G0[:, NKV * c2:NKV * (c2 + 1)]

            def wk(c2):   # [128, D]
                return G0[:, DC * NKV + D * c2:DC * NKV + D * (c2 + 1)]

            def xT(c2):   # [128, N]
                return G1[:, N * c2:N * (c2 + 1)]

            def wq(c2):   # [128, D]
                return G1[:, DC * N + D * c2:DC * N + D * (c2 + 1)]

            def wv(c2):
                return G2[:, D * c2:D * (c2 + 1)]

            def wo(c2):
                return G3[:, D * c2:D * (c2 + 1)]

            # persistent result tiles
            kT_t = [cpool.tile([128, NKV], bf16, name=f"kT{c}")
                    for c in range(DC)]
            qT_t = [cpool.tile([128, N], bf16, name=f"qT{c}")
                    for c in range(DC)]
            v_t = [cpool.tile([128, D], bf16, name=f"v{c}")
                   for c in range(KC)]
            aoT_t = [cpool.tile([128, N], bf16, name=f"aoT{c}")
                     for c in range(DC)]
            zeros = cpool.tile([128, 512], bf16, name="zeros")
            ones_kv = cpool.tile([128, 1], bf16, name="ones_kv")
            onesq = cpool.tile([1, 128], bf16, name="onesq")
            ones97 = cpool.tile([97, 64], bf16, name="ones97")
            dum = cpool.tile([1, 32], f32, name="dum")
            dbg = (cpool.tile([128, 512 * _NPROBE], f32, name="dbg")
                   if _DEBUG else None)

            def probe(k, src, rows=128, cols=512, row0=0):
                if _DEBUG:
                    r1 = row0 + 1 if rows is None else rows
                    nc.vector.tensor_scalar_add(
                        dbg[row0:r1, 512 * k:512 * k + cols], src, 0.0)

            # ---- issue everything up front ----
            nc.vector.memset(zeros[:], 0.0)
            nc.vector.memset(ones_kv[:], 1.0)
            nc.vector.memset(onesq[:], 1.0)
            nc.vector.memset(ones97[:], 1.0)
            if _DEBUG:
                nc.vector.memset(dbg[:], 0.0)
            nc.sync.dma_start(G0[:], g0_d[:, :])
            nc.sync.dma_start(G1[:], g1_d[:, :])
            nc.sync.dma_start(G2[:], g2_d[:, :])
            nc.sync.dma_start(G3[:], g3_d[:, :])
            nc.sync.dma_start(G4[:], g4_d[:, :])
            # preload the exp table set while DMA is in flight
            nc.scalar.activation(dum[:], zeros[0:1, 0:32], AF.Exp, scale=1.0)

            # ---- PE warmup: open the HAM clock gate during DMA flight ----
            with tc.tile_pool(name="wp", bufs=1, space="PSUM") as wp:
                wps = wp.tile([128, 512], f32, name="wps")
                for _ in range(32):
                    nc.tensor.matmul(wps[:, 0:128], zeros[:, 0:128], zeros[:, 0:128],
                                     start=True, stop=True)

            # ---- k projection (no bias: cancels in softmax) ----
            with tc.tile_pool(name="kpp", bufs=2, space="PSUM") as kpp:
                for dp in range(DC):
                    ps = kpp.tile([128, NKV], f32, name="kps")
                    for c2 in range(DC):
                        lhs = wk(c2)[:, 128 * dp:128 * (dp + 1)]
                        nc.tensor.matmul(
                            ps[:, 0:512], lhs, xkT(c2)[:, 0:512],
                            start=(c2 == 0), stop=(c2 == DC - 1))
                        nc.tensor.matmul(
                            ps[:, 512:NKV], lhs, xkT(c2)[:, 512:NKV],
                            start=(c2 == 0), stop=(c2 == DC - 1))
                    nc.scalar.copy(kT_t[dp][:], ps[:])

            # ---- q projection for head pair 0 (rest are fillers) ----
            with tc.tile_pool(name="qpp", bufs=2, space="PSUM") as qpp:
                qps = [qpp.tile([128, 512], f32, name="qps")
                       for hf in range(2)]
                for c2 in range(DC):
                    for hf in range(2):
                        nc.tensor.matmul(
                            qps[hf][:], wq(c2)[:, 0:128],
                            xT(c2)[:, 512 * hf:512 * (hf + 1)],
                            start=(c2 == 0), stop=(c2 == DC - 1))
                for hf in range(2):
                    nc.vector.tensor_scalar_add(
                        qT_t[0][:, 512 * hf:512 * (hf + 1)], qps[hf][:],
                        G4[:, G4_BQ:G4_BQ + 1])

            # ---- v projection for chunks 0..3 (c4 is a filler) ----
            with tc.tile_pool(name="vpp", bufs=2, space="PSUM") as vpp:
                for c in range(KC - 1):
                    ps = vpp.tile([128, 512], f32, name="vps")
                    for c2 in range(DC):
                        nc.tensor.matmul(
                            ps[:], xkT(c2)[:, 128 * c:128 * (c + 1)],
                            wv(c2)[:], start=(c2 == 0), stop=(c2 == DC - 1))
                    nc.vector.tensor_scalar_add(v_t[c][:], ps[:], 0.0)

            # ---- attention over head pairs, leftover proj as fillers ----
            with tc.tile_pool(name="scp", bufs=2, space="PSUM") as scp, \
                 tc.tile_pool(name="oap", bufs=2, space="PSUM") as oap, \
                 tc.tile_pool(name="dnp", bufs=1, space="PSUM") as dnp, \
                 tc.tile_pool(name="flp", bufs=1, space="PSUM") as flp, \
                 tc.tile_pool(name="pp", bufs=4) as pp, \
                 tc.tile_pool(name="rcp", bufs=4) as rcp:

                # filler units emit 2 matmuls per step so one step fits the
                # PE slack of a chunk slot; the final step adds the
                # PSUM->SBUF copy
                class FillV:
                    def __init__(self, c):
                        self.c, self.ps = c, None

                    def step(self, half):
                        if half == 0:
                            self.ps = flp.tile([128, 512], f32, name="fps")
                        for c2 in (0, 1) if half == 0 else (2, 3):
                            nc.tensor.matmul(
                                self.ps[:],
                                xkT(c2)[:, 128 * self.c:128 * (self.c + 1)],
                                wv(c2)[:], start=(c2 == 0), stop=(c2 == 3))
                        if half == 1:
                            nc.vector.tensor_scalar_add(
                                v_t[self.c][:], self.ps[:], 0.0)

                class FillQ:
                    def __init__(self, dp, hf):
                        self.dp, self.hf, self.ps = dp, hf, None

                    def step(self, half):
                        if half == 0:
                            self.ps = flp.tile([128, 512], f32, name="fps")
                        for c2 in (0, 1) if half == 0 else (2, 3):
                            nc.tensor.matmul(
                                self.ps[:],
                                wq(c2)[:, 128 * self.dp:128 * (self.dp + 1)],
                                xT(c2)[:, 512 * self.hf:512 * (self.hf + 1)],
                                start=(c2 == 0), stop=(c2 == 3))
                        if half == 1:
                            nc.vector.tensor_scalar_add(
                                qT_t[self.dp][:,
                                              512 * self.hf:512 * (self.hf + 1)],
                                self.ps[:],
                                G4[:, G4_BQ + self.dp:G4_BQ + self.dp + 1])

                v4 = FillV(4)
                q10, q11 = FillQ(1, 0), FillQ(1, 1)
                q20, q21 = FillQ(2, 0), FillQ(2, 1)
                q30, q31 = FillQ(3, 0), FillQ(3, 1)
                fillers = {
                    (0, 0): [(v4, 0), (v4, 1)],
                    (0, 1): [(q10, 0)], (0, 2): [(q10, 1)],
                    (0, 3): [(q11, 0)], (0, 4): [(q11, 1)],
                    (1, 0): [(q20, 0)], (1, 1): [(q20, 1)],
                    (1, 2): [(q21, 0)], (1, 3): [(q21, 1)],
                    (1, 4): [(q30, 0)],
                    (2, 0): [(q30, 1)], (2, 1): [(q31, 0)],
                    (2, 2): [(q31, 1)],
                }

                pending_norm = []
                for dp in range(DC):
                    oa = [oap.tile([128, 512], f32, name="oa")
                          for hf in range(2)]
                    den = dnp.tile([128, 512], f32, name="den")
                    p_t = []  # per chunk: [p_hf0, p_hf1]

                    def scores4(c):
                        # 4 matmuls sharing one kT weight pair: the two
                        # query halves reuse the loaded weights, the A/B
                        # heads run concurrently on disjoint PE row groups
                        sc = [scp.tile([128, N], f32, name="sc")
                              for hf in range(2)]
                        for hf in range(2):
                            for hi in range(2):
                                nc.tensor.matmul(
                                    sc[hf][:, 512 * hi:512 * (hi + 1)],
                                    kT_t[dp][HD * hi:HD * (hi + 1),
                                             128 * c:128 * (c + 1)],
                                    qT_t[dp][HD * hi:HD * (hi + 1),
                                             512 * hf:512 * (hf + 1)],
                                    start=True, stop=True)
                        ps = []
                        for hf in range(2):
                            p = pp.tile([128, N], bf16, name="p")
                            nc.scalar.activation(
                                p[:], sc[hf][:], AF.Exp,
                                bias=G4[:, G4_EXPB + c:G4_EXPB + c + 1],
                                scale=SCALE)
                            ps.append(p)
                        if _DEBUG and dp == 0 and c == 0:
                            probe(5, sc[0][:, 0:512])
                            probe(6, ps[0][:, 0:512])
                        return ps, sc

                    def av4den4(c):
                        # col-packed pairs: head A -> partitions 0:64, head
                        # B -> 64:128 of one PSUM bank per query half; both
                        # halves adjacent so the v weights load once. Each
                        # writer is its own accumulation group over c
                        # (per-partition zero regions) -> skip_group_check.
                        for hf in range(2):
                            for hi in range(2):
                                h = 2 * dp + hi
                                nc.tensor.matmul(
                                    oa[hf][64 * hi:64 * (hi + 1), :],
                                    v_t[c][:, 64 * h:64 * h + 64],
                                    p_t[c][hf][:, 512 * hi:512 * (hi + 1)],
                                    start=(c == 0), stop=(c == KC - 1),
                                    tile_position=(0, 64 * hi),
                                    skip_group_check=True)
                        # denominators: M=1 ones-matmuls on 4 distinct col
                        # groups run concurrently off one weight load
                        for hf in range(2):
                            for hi in range(2):
                                r = 64 * hf + 32 * hi
                                nc.tensor.matmul(
                                    den[r:r + 1, :], ones_kv[:],
                                    p_t[c][hf][:, 512 * hi:512 * (hi + 1)],
                                    start=(c == 0), stop=(c == KC - 1),
                                    tile_position=(0, r),
                                    skip_group_check=True)

                    def keep_warm(sc_dead, n):
                        # dead-tile dummy matmuls: keep the PE busy across
                        # cross-engine gate windows so the HAM clock gate
                        # stays open (micro-idle re-throttles cost 2x)
                        for _ in range(n):
                            nc.tensor.matmul(
                                sc_dead[:, 0:256], zeros[:, 0:128],
                                zeros[:, 0:256], start=True, stop=True)

                    sc_t = []
                    for c in range(KC):
                        pc, scc = scores4(c)
                        p_t.append(pc)
                        sc_t.append(scc)
                        if c == 0 and pending_norm:
                            keep_warm(prev_sc1, 10)
                            pending_norm.pop()()
                        if c >= 1:
                            av4den4(c - 1)
                        for unit, half in fillers.get((dp, c), ()):
                            unit.step(half)
                    av4den4(KC - 1)
                    prev_sc1 = sc_t[KC - 1][1]
                    if dp == DC - 1:
                        keep_warm(prev_sc1, 22)

                    # normalize: stage the four denominator rows into SBUF
                    # (same-partition copies; SBUF APs must start at
                    # 0/32/64/96), one reciprocal from partition 0 (custom
                    # DVE ops misbehave at non-zero start partitions), then
                    # partition-broadcast via col-packed K=1 matmuls
                    # (gpsimd partition_broadcast cannot write partitions
                    # 64:128 on HW) and one multiply per query half
                    def normalize(dp=dp, oa=oa, den=den):
                        db = rcp.tile([97, 512], f32, name="db")
                        rcf = rcp.tile([97, 512], f32, name="rcf")
                        rc = rcp.tile([97, 512], bf16, name="rc")
                        nc.vector.memset(db[:], 1.0)
                        for j in range(4):
                            r = 32 * j
                            nc.vector.tensor_scalar_add(
                                db[r:r + 1, :], den[r:r + 1, :], 0.0)
                        nc.vector.reciprocal_approx_fast(rcf[:], db[:])
                        nc.vector.tensor_scalar_add(rc[:], rcf[:], 0.0)
                        if _DEBUG and dp == 0:
                            for j in range(4):
                                r = 32 * j
                                probe(0, db[r:r + 1, :], rows=None, row0=r)
                                probe(1, rc[r:r + 1, :], rows=None, row0=r)
                        for hf in range(2):
                            rbs = flp.tile([128, 512], f32, name="fps")
                            for hi in range(2):
                                r = 64 * hf + 32 * hi
                                nc.tensor.matmul(
                                    rbs[64 * hi:64 * (hi + 1), :],
                                    ones97[r:r + 1, :], rc[r:r + 1, :],
                                    start=True, stop=True,
                                    tile_position=(r, 64 * hi),
                                    skip_group_check=True)
                            rbs_sb = rcp.tile([128, 512], f32, name="rbs_sb")
                            nc.vector.tensor_scalar_add(rbs_sb[:], rbs[:], 0.0)
                            if _DEBUG and dp == 0 and hf == 0:
                                probe(2, rbs_sb[:])
                                probe(3, oa[0][:])
                            nc.vector.tensor_mul(
                                aoT_t[dp][:, 512 * hf:512 * (hf + 1)],
                                oa[hf][:], rbs_sb[:])
                        if _DEBUG and dp == 0:
                            probe(4, aoT_t[0][:, 0:512])
                            probe(7, kT_t[0][:, 0:512])
                            probe(8, qT_t[0][:, 0:512])
                            probe(9, v_t[0][:, 0:512])

                    if dp == DC - 1:
                        normalize()
                    else:
                        pending_norm.append(normalize)

            # ---- output projection, bf16 writeback ----
            with tc.tile_pool(name="ypp", bufs=2, space="PSUM") as ypp, \
                 tc.tile_pool(name="ysp", bufs=2) as ysp:
                for ic in range(N // 128):
                    yps = ypp.tile([128, D], f32, name="yps")
                    for dp in range(DC):
                        nc.tensor.matmul(
                            yps[:], aoT_t[dp][:, 128 * ic:128 * (ic + 1)],
                            wo(dp)[:], start=(dp == 0), stop=(dp == DC - 1))
                    ysb = ysp.tile([128, D], bf16, name="ysb")
                    nc.vector.tensor_add(ysb[:], yps[:],
                                         G4[:, G4_BOB:G4_BOB + D])
                    nc.sync.dma_start(y_d[128 * ic:128 * (ic + 1), :], ysb[:])
                if _DEBUG:
                    nc.sync.dma_start(dbg_d[:, :], dbg[:])

    return nc


def _get_program():
    if "nc" not in _prog_cache:
        nc = _build_program()
        if not nc.is_finalized():
            nc.finalize()
        _prog_cache["nc"] = nc
    return _prog_cache["nc"]


def _prep_core(b, x, mask, wq, bq, wk, bk, wv, bv, wo, bo):
    import ml_dtypes

    b16 = ml_dtypes.bfloat16
    f = np.float32
    xb = np.ascontiguousarray(x[b], dtype=f)                # [N, D]
    idx = np.nonzero(mask[b])[0]
    nv = int(idx.size)
    assert 1 <= nv <= NKV, f"batch {b}: {nv} valid keys, NKV={NKV}"
    xk = np.zeros((NKV, D), f)
    xk[:nv] = xb[idx]
    xkT = np.ascontiguousarray(xk.T)                        # [D, NKV]
    xT = np.ascontiguousarray(xb.T)                         # [D, N]

    def chunks(a):  # [D, W] -> [128, DC*W]
        return np.concatenate([a[128 * c:128 * (c + 1), :]
                               for c in range(DC)], axis=1)

    g0 = np.concatenate([chunks(xkT), chunks(wk.astype(f))], axis=1)
    g1 = np.concatenate([chunks(xT), chunks(wq.astype(f))], axis=1)
    bobv = (bo.astype(f) + bv.astype(f) @ wo.astype(f)).reshape(1, D)
    g2 = np.concatenate([chunks(wv.astype(f)),
                         np.broadcast_to(bobv, (128, D))], axis=1)
    g3 = chunks(wo.astype(f))

    pos = np.arange(128)[:, None] + 128 * np.arange(KC)[None, :]
    expb = np.where(pos < nv, 0.0, PAD_BIAS).astype(f)      # [128, KC]
    bqc = np.stack([bq.astype(f)[128 * c:128 * (c + 1)]
                    for c in range(DC)], axis=1)            # [128, DC]
    bob = (bo.astype(f) + bv.astype(f) @ wo.astype(f)).reshape(D)
    g4 = np.concatenate([bqc, expb,
                         np.broadcast_to(bob, (128, D))], axis=1)
    return {
        "g0": np.ascontiguousarray(g0).astype(b16),
        "g1": np.ascontiguousarray(g1).astype(b16),
        "g2": np.ascontiguousarray(g2).astype(b16),
        "g3": np.ascontiguousarray(g3).astype(b16),
        "g4": np.ascontiguousarray(g4, f),
    }


def _run(inputs):
    import os

    os.environ["BASS_NEVER_TRACE"] = "1"
    from concourse.bass_utils import run_bass_kernel_spmd

    nc = _get_program()
    in_maps = [_prep_core(b, **inputs) for b in range(B)]
    res = run_bass_kernel_spmd(nc, in_maps, core_ids=list(range(B)),
                               trace=False)
    out = np.stack([res.results[b]["y"] for b in range(B)], axis=0)
    return out.astype(np.float32), res


def kernel(**inputs) -> np.ndarray:
    out, _ = _run(inputs)
    return out
